# revision 4
# baseline (speedup 1.0000x reference)
"""Trainium2 Bass kernel for nn_CustomLSTM: 1000-step LSTM, batch 128,
input 128, hidden 1024, 50 categories, on 8 NeuronCores (one trn2 chip).

Sharding: model-parallel over the hidden dimension (core p owns hidden block
p: 128 of 1024 units for all four gates), full batch 128 on the PE partition
axis so every recurrent matmul runs with a full 128x128 stationary.

Exchange: instead of the ncfw AllGather (~16.7us/step RTT through TOPSP
firmware + HBM bounce), each core pushes its bf16 hT slice [128, 128]
directly into all 8 cores' SBUF via remote_dma_broadcast (SWDGE -> SDMA,
SBUF->SBUF over RMTV/D2D). Completion is signalled by the broadcast's
remote-semaphore update (+2 per sender per step, so remote_sem >= 16*t
proves all 8 slices of step t have landed). The per-sender destination
offset (slot p) is baked in by wrapping the gpsimd loop in one Switch over
the runtime core id. Descriptors for step t+1 are prepared while step t's
transfer is in flight; double-buffered send/gather buffers keep senders
(which may run one step ahead) from clobbering unconsumed data.

kernel(**inputs) takes the FULL unsharded inputs keyed as in setup_inputs()
and returns the FULL [128, 50] float32 output.
"""

from contextlib import ExitStack

import numpy as np
import ml_dtypes

from concourse import bass, mybir
from concourse.bass import _bass_rust
from concourse.bass_utils import run_bass_kernel_spmd
from concourse.library_config import all_libraries as _all_libs, standard as _std_lib

N_CORES = 8
B = 128      # batch
H = 1024     # hidden
HP = H // N_CORES
NG = 4 * HP  # gate columns per core, order f|i|o|g
S = 1000     # sequence length
I = 128      # input features
F32 = mybir.dt.float32
BF16 = mybir.dt.bfloat16
SIG = mybir.ActivationFunctionType.Sigmoid
TANH = mybir.ActivationFunctionType.Tanh
RDESTS = [(0, k) for k in range(N_CORES)]


def _finish_extended(nc):
    """Raw Bass skips Bacc's library-load + extended-inst codegen passes;
    remote_dma* need both (else walrus rejects the empty .instr bytes)."""
    mask = {}
    for lib in _all_libs:
        for inst_type in lib.instructions:
            mask[inst_type] = mask.get(inst_type, 0) | (1 << lib.index)
    _bass_rust.insert_library_loads(nc, mask, len(_all_libs), _std_lib.index)
    mybir.codegen_inst_isa_subclasses(nc)
    return nc


def _build_lstm(seq_len=S, xt_depth: int = 8):
    nc = bass.Bass(num_devices=N_CORES, target_bir_lowering=False, debug=False)
    SL = seq_len

    xT = nc.declare_dram_parameter("xT", [SL, I, B], F32, isOutput=False)
    wh = nc.declare_dram_parameter("wh", [H, NG], BF16, isOutput=False)
    wx = nc.declare_dram_parameter("wx", [I, NG], F32, isOutput=False)
    brow = nc.declare_dram_parameter("brow", [1, NG], F32, isOutput=False)
    wout = nc.declare_dram_parameter("wout", [HP, 50], F32, isOutput=False)
    ident = nc.declare_dram_parameter("ident", [128, 128], F32, isOutput=False)
    ones = nc.declare_dram_parameter("ones", [1, B], F32, isOutput=False)
    y = nc.declare_dram_parameter("y", [B, 50], F32, isOutput=True)

    with ExitStack() as _es:
        wh_sb = _es.enter_context(nc.sbuf_tensor("wh_sb", [128, 8 * NG], BF16))
        wx_sb = _es.enter_context(nc.sbuf_tensor("wx_sb", [128, NG], F32))
        b_sb = _es.enter_context(nc.sbuf_tensor("b_sb", [1, NG], F32))
        wout_sb = _es.enter_context(nc.sbuf_tensor("wout_sb", [128, 50], F32))
        id_sb = _es.enter_context(nc.sbuf_tensor("id_sb", [128, 128], F32))
        ones_sb = _es.enter_context(nc.sbuf_tensor("ones_sb", [1, B], F32))
        xt_sb = _es.enter_context(
            nc.sbuf_tensor("xt_sb", [128, xt_depth * B], F32)
        )
        # C = 2c; gates arrive pre-scaled so one tanh yields
        # tf=tanh(zf/2) etc. with sigma(z) = (tanh(z/2)+1)/2
        c_sb = _es.enter_context(nc.sbuf_tensor("c_sb", [128, HP], F32))
        t_sb = _es.enter_context(nc.sbuf_tensor("t_sb", [128, NG], F32))
        # double-buffered gather target: buffer t%2 holds h(t)
        hT_g0 = _es.enter_context(nc.sbuf_tensor("hT_g0", [128, H], BF16))
        hT_g1 = _es.enter_context(nc.sbuf_tensor("hT_g1", [128, H], BF16))
        # double-buffered send staging (own hT slice, bf16)
        hT_s0 = _es.enter_context(nc.sbuf_tensor("hT_s0", [128, B], BF16))
        hT_s1 = _es.enter_context(nc.sbuf_tensor("hT_s1", [128, B], BF16))
        hT_fin = _es.enter_context(nc.sbuf_tensor("hT_fin", [128, B], F32))
        tf_sb = t_sb[:, 0:HP]
        ti_sb = t_sb[:, HP : 2 * HP]
        to_sb = t_sb[:, 2 * HP : 3 * HP]
        tg4_sb = t_sb[:, 3 * HP : 4 * HP]
        fc_sb = _es.enter_context(nc.sbuf_tensor("fc_sb", [128, HP], F32))
        ig_sb = _es.enter_context(nc.sbuf_tensor("ig_sb", [128, HP], F32))
        h_sb = _es.enter_context(nc.sbuf_tensor("h_sb", [128, HP], F32))
        tc_sb = _es.enter_context(nc.sbuf_tensor("tc_sb", [128, HP], F32))
        y_sb = _es.enter_context(nc.sbuf_tensor("y_sb", [128, 50], F32))
        g_ps0 = _es.enter_context(nc.psum_tensor("g_ps0", [128, NG], F32))
        g_ps1 = _es.enter_context(nc.psum_tensor("g_ps1", [128, NG], F32))
        hT_ps = _es.enter_context(nc.psum_tensor("hT_ps", [128, B], F32))
        y_ps = _es.enter_context(nc.psum_tensor("y_ps", [128, 50], F32))
        dma_w_sem = _es.enter_context(nc.semaphore("dma_w_sem"))
        dma_x_sem = _es.enter_context(nc.semaphore("dma_x_sem"))
        dma_y_sem = _es.enter_context(nc.semaphore("dma_y_sem"))
        # per-slot remote sems: sender p increments rsems[p] on every core
        # (+2 per broadcast); receivers gate matmul k on rsems[k] >= 2*(t-1)
        rsems = [
            _es.enter_context(nc.semaphore(f"rsem{k}")) for k in range(N_CORES)
        ]
        lsem = _es.enter_context(nc.semaphore("lsem"))
        psem = _es.enter_context(nc.semaphore("psem"))
        pe_g_sem = _es.enter_context(nc.semaphore("pe_g_sem"))
        pe_x_sem = _es.enter_context(nc.semaphore("pe_x_sem"))
        pe_tr_sem = _es.enter_context(nc.semaphore("pe_tr_sem"))
        act_sem = _es.enter_context(nc.semaphore("act_sem"))
        dve_c_sem = _es.enter_context(nc.semaphore("dve_c_sem"))
        dve_h_sem = _es.enter_context(nc.semaphore("dve_h_sem"))
        dve_hT_sem = _es.enter_context(nc.semaphore("dve_hT_sem"))
        dve_y_sem = _es.enter_context(nc.semaphore("dve_y_sem"))
        init_sem = _es.enter_context(nc.semaphore("init_sem"))
        block = _es.enter_context(nc.Block())
        g_ps = [g_ps0, g_ps1]
        hT_g = [hT_g0, hT_g1]
        hT_s = [hT_s0, hT_s1]

        def xt_tile(t):
            s = (t - 1) % xt_depth
            return xt_sb[:, s * B : (s + 1) * B]

        def wh_tile(k):
            return wh_sb[:, k * NG : (k + 1) * NG]

        N_INIT = 13

        # ---------------- sync engine: init + xT prefetch + y out ----------
        @block.sync
        def _(sync):
            for k in range(8):
                sync.dma_start(
                    out=wh_tile(k), in_=wh[k * 128 : (k + 1) * 128, :]
                ).then_inc(dma_w_sem, 16)
            sync.dma_start(out=wx_sb[:, :], in_=wx[:, :]).then_inc(dma_w_sem, 16)
            sync.dma_start(out=b_sb[:, :], in_=brow[:, :]).then_inc(dma_w_sem, 16)
            sync.dma_start(out=wout_sb[0:HP, :], in_=wout[:, :]).then_inc(
                dma_w_sem, 16
            )
            sync.dma_start(out=id_sb[:, :], in_=ident[:, :]).then_inc(
                dma_w_sem, 16
            )
            sync.dma_start(out=ones_sb[:, :], in_=ones[:, :]).then_inc(
                dma_w_sem, 16
            )
            for t in range(1, min(xt_depth, SL) + 1):
                if t >= 2:
                    sync.wait_ge(dma_x_sem, 16 * (t - 1))
                sync.dma_start(out=xt_tile(t), in_=xT[t - 1]).then_inc(
                    dma_x_sem, 16
                )
            for t in range(1, SL + 1):
                j = t + xt_depth
                if j <= SL:
                    sync.wait_ge(pe_x_sem, j - xt_depth)
                    sync.wait_ge(dma_x_sem, 16 * (j - 1))
                    sync.dma_start(out=xt_tile(j), in_=xT[j - 1]).then_inc(
                        dma_x_sem, 16
                    )
            sync.wait_ge(dve_y_sem, 1)
            sync.dma_start(out=y[:, :], in_=y_sb[0:B, :]).then_inc(
                dma_y_sem, 16
            )
            sync.wait_ge(dma_y_sem, 16)

        # -------- gpsimd: init + per-step broadcast of own hT slice --------
        @block.gpsimd
        def _(g):
            g.memset(c_sb[:, :], 0.0).then_inc(init_sem, 1)
            if SL < 2:
                return
            pid = g.partition_id()
            for p in g.Switch(pid, N_CORES):
                sl = slice(p * B, (p + 1) * B)
                g.remote_dma_broadcast(
                    out_ap=hT_g[1][:, sl],
                    in_ap=hT_s[1][:, :],
                    remote_sem=rsems[p],
                    local_sem=lsem,
                    rdests=RDESTS,
                ).then_inc(psem, 1)
                for t in range(1, SL):
                    g.wait_ge(dve_hT_sem, t)
                    g.wait_ge(psem, t)
                    g.trigger_dma(1)
                    if t + 1 < SL:
                        g.remote_dma_broadcast(
                            out_ap=hT_g[(t + 1) % 2][:, sl],
                            in_ap=hT_s[(t + 1) % 2][:, :],
                            remote_sem=rsems[p],
                            local_sem=lsem,
                            rdests=RDESTS,
                        ).then_inc(psem, 1)

        # ---------------- PE ----------------
        @block.tensor
        def _(tensor):
            tensor.wait_ge(dma_w_sem, 16 * N_INIT)

            def xbias(t, stop=False):
                bank = g_ps[t % 2]
                tensor.wait_ge(dma_x_sem, 16 * t)
                tensor.matmul(
                    bank[:, :], xt_tile(t), wx_sb[:, :], start=True, stop=False
                ).then_inc(pe_x_sem, 1)
                return tensor.matmul(
                    bank[:, :], ones_sb[0:1, :], b_sb[0:1, :],
                    start=False, stop=stop,
                )

            def warmer():
                # tiny matmul into the (otherwise unused until the end) y_ps
                # bank: keeps the PE HAM activity window from seeing a full
                # ~3.4us idle stretch mid-step, which would re-throttle the
                # PE clock to 1.2 GHz for the next step's matmuls
                tensor.matmul(
                    y_ps[:, :], ones_sb[0:1, :], b_sb[0:1, 0:50],
                    start=True, stop=True,
                )

            xbias(1, stop=True).then_inc(pe_g_sem, 1)
            if SL >= 2:
                xbias(2)
            for t in range(2, SL + 1):
                # mid-chain warmer: fires once tanh(gates) of t-1 is done
                # (keeps the PE HAM window active; the wait is satisfied
                # well before dve_h below, so it is off the critical path)
                tensor.wait_ge(act_sem, 2 * (t - 2) + 1)
                warmer()
                # transpose h(t-1) for the exchange
                tensor.wait_ge(dve_h_sem, t - 1)
                tensor.transpose(hT_ps[:, :], h_sb[:, :], id_sb[:, :]).then_inc(
                    pe_tr_sem, 1
                )
                # recurrent matmuls, each gated on its own slot's arrival
                buf = hT_g[(t - 1) % 2]
                for k in range(8):
                    tensor.wait_ge(rsems[k], 2 * (t - 1))
                    mm = tensor.matmul(
                        g_ps[t % 2][:, :], buf[:, k * B : (k + 1) * B],
                        wh_tile(k), start=False, stop=(k == 7),
                    )
                    if k == 7:
                        mm.then_inc(pe_g_sem, 1)
                # x-projection for t+1 runs in PE idle time, off the
                # critical path (next consumer is mms(t+1))
                if t + 1 <= SL:
                    xbias(t + 1)
            tensor.wait_ge(dve_h_sem, SL)
            tensor.transpose(hT_ps[:, :], h_sb[:, :], id_sb[:, :]).then_inc(
                pe_tr_sem, 1
            )
            tensor.wait_ge(dve_hT_sem, SL)
            tensor.matmul(
                y_ps[:, :], hT_fin[:, :], wout_sb[:, :], start=True, stop=True
            ).then_inc(pe_g_sem, 1)

        # ------- ACT (scalar): activations only -------
        # act_sem per step: +1 tanh(all gates), +2 tanh(C/2)
        @block.scalar
        def _(act):
            for t in range(1, SL + 1):
                bank = g_ps[t % 2]
                act.wait_ge(pe_g_sem, t)
                act.activation(t_sb[:, :], bank[:, :], TANH).then_inc(
                    act_sem, 1
                )
                act.wait_ge(dve_c_sem, t)
                act.activation(
                    tc_sb[:, :], c_sb[:, :], TANH, scale=0.5
                ).then_inc(act_sem, 1)

        # ---------------- DVE (vector): cell update + copies ----------------
        @block.vector
        def _(v):
            v.wait_ge(init_sem, 1)
            ADD = mybir.AluOpType.add
            MUL = mybir.AluOpType.mult
            for t in range(1, SL + 1):
                base = 2 * (t - 1)
                v.wait_ge(act_sem, base + 1)
                # C = 2c recurrence: C = 0.5*(tf+1)*C + (ti+1)*tg
                v.scalar_tensor_tensor(
                    fc_sb[:, :], tf_sb, 1.0, c_sb[:, :], ADD, MUL
                )
                v.scalar_tensor_tensor(
                    ig_sb[:, :], ti_sb, 1.0, tg4_sb, ADD, MUL
                )
                v.scalar_tensor_tensor(
                    c_sb[:, :], fc_sb[:, :], 0.5, ig_sb[:, :], MUL, ADD
                ).then_inc(dve_c_sem, 1)
                v.wait_ge(act_sem, base + 2)
                # H = 2h = (to+1)*tanh(c); the 1/2 is folded into wh/wout
                v.scalar_tensor_tensor(
                    h_sb[:, :], to_sb, 1.0, tc_sb[:, :], ADD, MUL
                ).then_inc(dve_h_sem, 1)
                if t <= SL - 1:
                    v.wait_ge(pe_tr_sem, t)
                    if t >= 3:
                        # step t-2's send from this buffer must be on the wire
                        v.wait_ge(lsem, 16 * (t - 2))
                    v.tensor_copy(hT_s[t % 2][:, :], hT_ps[:, :]).then_inc(
                        dve_hT_sem, 1
                    )
            # fp32 copy of hT_S for the fp32 output projection
            v.wait_ge(pe_tr_sem, SL)
            v.tensor_copy(hT_fin[:, :], hT_ps[:, :]).then_inc(dve_hT_sem, 1)
            v.wait_ge(pe_g_sem, SL + 1)
            v.tensor_copy(y_sb[:, :], y_ps[:, :]).then_inc(dve_y_sem, 1)

    return _finish_extended(nc)


def _prep_inputs(x, W_ii, W_hi, b_ii, W_if, W_hf, b_if, W_ig, W_hg, b_ig,
                 W_io, W_ho, b_io, W_out, b_out):
    """Per-core inputs. Gate column order f|i|o|g (sigmoid block contiguous);
    core p gets hidden slice [p*128, (p+1)*128) of every gate."""
    x = np.ascontiguousarray(np.asarray(x, np.float32))
    xT = np.ascontiguousarray(np.transpose(x, (1, 2, 0)))  # [S, I, B]

    Wx_gates = [W_if, W_ii, W_io, W_ig]
    Wh_gates = [W_hf, W_hi, W_ho, W_hg]
    b_gates = [b_if, b_ii, b_io, b_ig]

    ident = np.eye(128, dtype=np.float32)
    ones_row = np.ones((1, B), np.float32)

    in_maps = []
    for p in range(N_CORES):
        sl = slice(p * HP, (p + 1) * HP)
        # sigma(z) = (tanh(z/2)+1)/2: halve the f,i,o gate pre-activations
        # (columns 0:3HP) so one tanh over all gates gives tf,ti,to,tg.
        # The recurrent state travels as H = 2h, so wh picks up another
        # global 1/2, and wout as well.
        wx = np.concatenate(
            [np.asarray(w, np.float32)[:, sl] for w in Wx_gates], axis=1
        )
        wx[:, 0 : 3 * HP] *= 0.5
        whf = np.concatenate(
            [np.asarray(w, np.float32)[:, sl] for w in Wh_gates], axis=1
        )
        whf *= 0.5
        whf[:, 0 : 3 * HP] *= 0.5
        whm = whf.astype(ml_dtypes.bfloat16)
        brow = np.concatenate(
            [np.asarray(b, np.float32)[sl] for b in b_gates]
        )[None, :]
        brow[:, 0 : 3 * HP] *= 0.5
        woutT = np.ascontiguousarray(
            np.asarray(W_out, np.float32)[:, sl].T * 0.5
        )
        in_maps.append(
            dict(
                xT=xT,
                wh=np.ascontiguousarray(whm),
                wx=np.ascontiguousarray(wx),
                brow=np.ascontiguousarray(brow),
                wout=woutT,
                ident=ident,
                ones=ones_row,
            )
        )
    return in_maps


_CACHED = {}


def _get_nc():
    if "nc" not in _CACHED:
        _CACHED["nc"] = _build_lstm()
    return _CACHED["nc"]


def kernel(**inputs) -> np.ndarray:
    inputs = {k: np.asarray(v) for k, v in inputs.items()}
    in_maps = _prep_inputs(**inputs)
    nc = _get_nc()
    res = run_bass_kernel_spmd(nc, in_maps, core_ids=list(range(N_CORES)))
    y = sum(np.asarray(r["y"], np.float64) for r in res.results)
    y = y + np.asarray(inputs["b_out"], np.float64)
    return y.astype(np.float32)
